# revision 12
# baseline (speedup 1.0000x reference)
"""ChessGNN (2-layer GCN + global max pool + FC + log_softmax) on 8 Trainium2 cores.

v2 strategy (edge-parallel, dst-range sharded):
  - deg/dinv depend only on edge_index -> host-computed. dinv is folded into
    x on host (xs = x * dinv), so the layer-1 message table hs1 = xs @ W1 can
    be computed *locally in full* by every core (x is replicated): the first
    AllGather is eliminated.
  - Core k owns dst nodes [12500k, 12500(k+1)). Edges routed to the core
    owning their dst, ordered (src-range, dst-bucket); bucket = 128 dst rows.
  - Messages fetched with gpsimd dma_gather (4096 idxs/instruction, int16
    idxs => 4 src ranges of 32768 rows) from a [100352, 64]-stride f32 table
    (cols 0:32 live).
  - Segment-sum via one-hot matmul: per 128-edge chunk a [128e,128slot]
    one-hot (DVE is_equal vs iota) x [128e,32] messages on PE. Each bucket
    accumulates in its own persistent PSUM tile across all ranges (98 tiles
    fit PSUM), one Act-engine dump per bucket.
  - Self-loop term via an own-slice input (xsT_own): hs_slice kept in SBUF,
    combine h = relu(dinv*(acc + hs) + b).
  - Layer 2: hs2 = (dinv*h1) @ W2 via PE transposes, AllGather (Shared out)
    -> same gather/aggregate.
  - Head: local max -> PE transpose -> AllReduce(max) -> FC -> log_softmax.
"""
import numpy as np

import concourse.bass as bass
import concourse.bacc as bacc
import concourse.mybir as mybir
import concourse.tile as tile
from concourse.bass_utils import run_bass_kernel_spmd
from concourse.masks import make_identity

N = 100000
NCORES = 8
S = N // NCORES            # 12500 nodes per core
NB = 98                    # buckets of 128 dst nodes (98*128 = 12544)
SP = NB * 128              # padded slice rows
NPAD = NCORES * SP         # padded global rows = 100352
NT = NPAD // 128           # 784 tiles of 128 nodes
RNG = 32768                # int16 gather range
NRANGES = (NPAD + RNG - 1) // RNG  # 4
PADDLOC = 999.0
CPI = 32                   # chunks per gather instruction (4096 idxs)

LAST_RESULTS = None
RUN_WALL_NS = None


def _host_prep(x, src, dst):
    """Index marshaling: per-core gather indices, dloc tables, chunk schedule."""
    deg = np.bincount(dst, minlength=N).astype(np.float32) + 1.0  # + self loop
    dinv = 1.0 / np.sqrt(deg)
    xs = np.asarray(x, np.float32) * dinv[:, None]

    # padded global numbering: node n -> (n//S)*SP + n%S
    src_gid = (src // S) * SP + (src % S)

    # xs in padded layout, transposed: [8, NPAD]
    xsp = np.zeros((NPAD, 8), np.float32)
    rows = (np.arange(N) // S) * SP + (np.arange(N) % S)
    xsp[rows] = xs
    xsT = np.ascontiguousarray(xsp.T)

    # per-core dinv of own slice in [128, NB] layout (partition=slot, col=bucket)
    dinv_sl = []
    for k in range(NCORES):
        d = np.ones(SP, np.float32)
        d[:S] = dinv[k * S:(k + 1) * S]
        dinv_sl.append(np.ascontiguousarray(d.reshape(NB, 128).T))

    # per-core own-slice xsT: [8, SP]
    xsT_own = [np.ascontiguousarray(xsT[:, k * SP:(k + 1) * SP]) for k in range(NCORES)]

    # ---- edge shards ----
    owner = dst // S
    per = []
    cnt = np.zeros((NCORES, NRANGES, NB), np.int64)
    for k in range(NCORES):
        m = owner == k
        sg = src_gid[m]
        dl = dst[m] - k * S
        rg = sg >> 15
        b = dl >> 7
        order = np.lexsort((b, rg))
        sg, dl, rg, b = sg[order], dl[order], rg[order], b[order]
        np.add.at(cnt[k], (rg, b), 1)
        per.append((sg, dl))

    chunks = (cnt.max(axis=0) + 127) // 128          # [NRANGES, NB]
    for b in range(NB):                              # every bucket gets >=1 chunk
        if chunks[:, b].sum() == 0:
            chunks[0, b] = 1
    cg = chunks.sum(axis=1)                          # chunks per range
    C = int(cg.sum())

    # processing order: range-major, bucket, chunk. flags per (g,b) run:
    # run_first/run_last bound the PSUM accumulation; bucket_first marks the
    # bucket's first run overall (dump = copy vs add into T).
    meta = []                                        # (b, run_first, run_last, bkt_first)
    seen = np.zeros(NB, np.int64)
    for g in range(NRANGES):
        for b in range(NB):
            nch = int(chunks[g, b])
            for c in range(nch):
                meta.append((b, c == 0, c == nch - 1, seen[b] == 0))
            if nch:
                seen[b] += 1

    # gather instructions: per range, CPI-chunk instrs + remainder
    instrs = []                                      # (g, chunk0, nch)
    c0 = 0
    for g in range(NRANGES):
        left = int(cg[g])
        while left > 0:
            nch = min(CPI, left)
            instrs.append((g, c0, nch))
            c0 += nch
            left -= nch
    GC = C * 8                                       # g16 cols (128 idx -> 8 cols)

    # per-core index/dloc arrays
    g16_all, dlf_all = [], []
    for k in range(NCORES):
        sg, dl = per[k]
        gidx = np.zeros(C * 128, np.int16)
        dloc = np.full(C * 128, PADDLOC, np.float32)
        ptr = 0
        pos = 0
        for g in range(NRANGES):
            for b in range(NB):
                n = int(cnt[k, g, b])
                cap = int(chunks[g, b]) * 128
                sl = slice(ptr, ptr + n)
                gidx[pos:pos + n] = (sg[sl] - g * RNG).astype(np.int16)
                dloc[pos:pos + n] = (dl[sl] - b * 128).astype(np.float32)
                ptr += n
                pos += cap
        # g16: per instr, idx i -> [i%16, i//16], replicated to 128 partitions
        g16 = np.zeros((16, GC), np.int16)
        col = 0
        for (g, ic0, nch) in instrs:
            arr = gidx[ic0 * 128:(ic0 + nch) * 128]
            g16[:, col:col + nch * 8] = arr.reshape(-1, 16).T
            col += nch * 8
        g16_all.append(np.ascontiguousarray(np.tile(g16, (8, 1))))
        dlf_all.append(np.ascontiguousarray(dloc.reshape(C, 128).T))

    return xsT, xsT_own, dinv_sl, g16_all, dlf_all, chunks, meta, instrs, C, GC


def build(x, edge_index, W1, b1, W2, b2, fcW, fcb):
    ei = np.asarray(edge_index)
    src = ei[0].astype(np.int64)
    dst = ei[1].astype(np.int64)

    (xsT, xsT_own, dinv_sl, g16_all, dlf_all, chunks, meta, instrs, C, GC
     ) = _host_prep(x, src, dst)

    iota = np.tile(np.arange(128, dtype=np.float32), (128, 1))
    b1t = np.tile(np.asarray(b1, np.float32)[None, :], (128, 1))
    b2t = np.tile(np.asarray(b2, np.float32)[None, :], (128, 1))
    fcb2 = np.asarray(fcb, np.float32)[None, :]

    # ---- SPMD program ----
    nc = bacc.Bacc("TRN2", target_bir_lowering=False, debug=False, num_devices=NCORES)
    dt = mybir.dt
    xsT_t = nc.dram_tensor("xsT", [8, NPAD], dt.float32, kind="ExternalInput")
    xso_t = nc.dram_tensor("xso", [8, SP], dt.float32, kind="ExternalInput")
    g16_t = nc.dram_tensor("g16", [128, GC], dt.int16, kind="ExternalInput")
    dlf_t = nc.dram_tensor("dlf", [128, C], dt.float32, kind="ExternalInput")
    dnv_t = nc.dram_tensor("dnv", [128, NB], dt.float32, kind="ExternalInput")
    iota_t = nc.dram_tensor("iota", [128, 128], dt.float32, kind="ExternalInput")
    W1_t = nc.dram_tensor("W1", [8, 32], dt.float32, kind="ExternalInput")
    W2_t = nc.dram_tensor("W2", [32, 32], dt.float32, kind="ExternalInput")
    b1_t = nc.dram_tensor("b1t", [128, 32], dt.float32, kind="ExternalInput")
    b2_t = nc.dram_tensor("b2t", [128, 32], dt.float32, kind="ExternalInput")
    fcW_t = nc.dram_tensor("fcW", [32, 5], dt.float32, kind="ExternalInput")
    fcb_t = nc.dram_tensor("fcb", [1, 5], dt.float32, kind="ExternalInput")
    out_t = nc.dram_tensor("out", [1, 5], dt.float32, kind="ExternalOutput")

    AF = mybir.ActivationFunctionType
    ALU = mybir.AluOpType
    AX = mybir.AxisListType

    with tile.TileContext(nc) as tc:
        with (
            tc.tile_pool(name="per", bufs=1) as per_p,
            tc.tile_pool(name="xb", bufs=2) as xb_p,
            tc.tile_pool(name="hb", bufs=2) as hb_p,
            tc.tile_pool(name="gt", bufs=3) as gt_p,
            tc.tile_pool(name="oh", bufs=4) as oh_p,
            tc.tile_pool(name="tr", bufs=2) as tr_p,
            tc.tile_pool(name="psa", bufs=2, space="PSUM") as psa_p,
            tc.tile_pool(name="psb", bufs=3, space="PSUM") as psb_p,
            tc.tile_pool(name="dram", bufs=1, space="DRAM") as dram_p,
        ):
            G16 = per_p.tile([128, GC], dt.int16)
            DLF = per_p.tile([128, C], dt.float32)
            IO = per_p.tile([128, 128], dt.float32)
            DINV = per_p.tile([128, NB], dt.float32)
            W1s = per_p.tile([8, 32], dt.float32)
            W2s = per_p.tile([32, 32], dt.float32)
            B1 = per_p.tile([128, 32], dt.float32)
            B2 = per_p.tile([128, 32], dt.float32)
            FCW = per_p.tile([32, 5], dt.float32)
            FCB = per_p.tile([1, 5], dt.float32)
            IDN = per_p.tile([128, 128], dt.float32)
            HS = per_p.tile([128, NB, 32], dt.float32)   # own-slice hs (layer 1/2)
            T = per_p.tile([128, NB, 32], dt.float32)    # agg accumulator / h
            HS2 = per_p.tile([128, NB, 32], dt.float32)

            for t_, s_ in ((G16, g16_t), (DLF, dlf_t), (IO, iota_t), (DINV, dnv_t),
                           (W1s, W1_t), (W2s, W2_t), (B1, b1_t), (B2, b2_t),
                           (FCW, fcW_t), (FCB, fcb_t)):
                nc.sync.dma_start(t_[:], s_[:, :])
            make_identity(nc, IDN[:])

            table1 = dram_p.tile([NPAD, 64], dt.float32)
            agin2 = dram_p.tile([SP, 64], dt.float32)
            agout2 = dram_p.tile([NPAD, 64], dt.float32, addr_space="Shared")
            arin = dram_p.tile([32, 1], dt.float32)
            arout = dram_p.tile([32, 1], dt.float32)

            dinv_b = DINV[:].rearrange("p (b o) -> p b o", o=1).to_broadcast([128, NB, 32])
            bias_b = lambda Bt: Bt[:].rearrange("p (o f) -> p o f", o=1) \
                                     .to_broadcast([128, NB, 32])

            # ---- table1 = xs @ W1 for ALL nodes (local, replicated work) ----
            HT = NB // 2                     # 49 tiles per half-block
            HC = HT * 128                    # 6272 nodes
            for hblk in range(2 * NCORES):
                XB = xb_p.tile([8, HC], dt.float32, tag="xb")
                nc.sync.dma_start(XB[:], xsT_t[:, hblk * HC:(hblk + 1) * HC])
                HB = hb_p.tile([128, HT, 32], dt.float32, tag="hb")
                for grp in range((HT + 7) // 8):
                    t0 = grp * 8
                    ng = min(8, HT - t0)
                    psg = psa_p.tile([128, 8, 32], dt.float32, tag="ptbl")
                    for i in range(ng):
                        nc.tensor.matmul(psg[:, i, :],
                                         lhsT=XB[:, (t0 + i) * 128:(t0 + i + 1) * 128],
                                         rhs=W1s[:], start=True, stop=True)
                    nc.scalar.copy(HB[:, t0:t0 + ng, :], psg[:, 0:ng, :])
                nc.sync.dma_start(
                    table1[hblk * HC:(hblk + 1) * HC, :]
                    .rearrange("(a p) b -> p a b", p=128)[:, :, 0:32], HB[:])

            # ---- own-slice hs (self-loop term) ----
            for oh2 in range(2):
                XB = xb_p.tile([8, HC], dt.float32, tag="xb")
                nc.sync.dma_start(XB[:], xso_t[:, oh2 * HC:(oh2 + 1) * HC])
                for grp in range((HT + 7) // 8):
                    t0 = grp * 8
                    ng = min(8, HT - t0)
                    psg = psa_p.tile([128, 8, 32], dt.float32, tag="ptbl")
                    for i in range(ng):
                        nc.tensor.matmul(psg[:, i, :],
                                         lhsT=XB[:, (t0 + i) * 128:(t0 + i + 1) * 128],
                                         rhs=W1s[:], start=True, stop=True)
                    nc.scalar.copy(HS[:, oh2 * HT + t0:oh2 * HT + t0 + ng, :],
                                   psg[:, 0:ng, :])

            def aggregate(table):
                """gather + one-hot matmul segment sum; PSUM accum per (g,b)
                run; dump = Act copy (first run) or DVE add into T."""
                psb = None
                for (g, ic0, nch) in instrs:
                    r0 = g * RNG
                    r1 = min((g + 1) * RNG, NPAD)
                    gt = gt_p.tile([128, CPI, 64], dt.float32, tag="gt")
                    nc.gpsimd.dma_gather(gt[:, 0:nch, :], table[r0:r1, :],
                                         G16[:, ic0 * 8:(ic0 + nch) * 8],
                                         nch * 128, nch * 128, 64)
                    done = 0
                    while done < nch:
                        gn = min(8, nch - done)
                        j0 = ic0 + done
                        oh = oh_p.tile([128, 8, 128], dt.float32, tag="oha")
                        nc.vector.tensor_tensor(
                            out=oh[:, 0:gn, :],
                            in0=DLF[:, j0:j0 + gn].rearrange("p (c o) -> p c o", o=1)
                                .to_broadcast([128, gn, 128]),
                            in1=IO[:].rearrange("p (o s) -> p o s", o=1)
                                .to_broadcast([128, gn, 128]),
                            op=ALU.is_equal)
                        for s in range(gn):
                            b, rfirst, rlast, bfirst = meta[j0 + s]
                            if rfirst:
                                psb = psb_p.tile([128, 32], dt.float32, tag="pagg",
                                                 name="pagg")
                            nc.tensor.matmul(psb[:], lhsT=oh[:, s, :],
                                             rhs=gt[:, done + s, 0:32],
                                             start=rfirst, stop=rlast)
                            if rlast:
                                if bfirst:
                                    nc.scalar.copy(T[:, b, :], psb[:])
                                else:
                                    nc.vector.tensor_add(T[:, b, :], T[:, b, :],
                                                         psb[:])
                        done += gn

            # ---- layer 1 ----
            aggregate(table1)
            nc.vector.tensor_add(T[:], T[:], HS[:])
            nc.vector.tensor_mul(T[:], T[:], dinv_b)
            nc.vector.tensor_add(T[:], T[:], bias_b(B1))
            nc.scalar.activation(T[:], T[:], AF.Relu)

            # ---- layer 2 prep: hs2 = (dinv*h1) @ W2 ----
            nc.vector.tensor_mul(T[:], T[:], dinv_b)
            for grp in range((NB + 7) // 8):
                t0 = grp * 8
                ng = min(8, NB - t0)
                psg = psa_p.tile([128, 8, 32], dt.float32, tag="ptbl")
                for i in range(ng):
                    pst = psa_p.tile([32, 128], dt.float32, tag="ptr", bufs=1)
                    nc.tensor.transpose(out=pst[:], in_=T[:, t0 + i, :], identity=IDN[:])
                    h1t = tr_p.tile([32, 128], dt.float32, tag="h1t")
                    nc.scalar.copy(h1t[:], pst[:])
                    nc.tensor.matmul(psg[:, i, :], lhsT=h1t[:], rhs=W2s[:],
                                     start=True, stop=True)
                nc.scalar.copy(HS2[:, t0:t0 + ng, :], psg[:, 0:ng, :])
            nc.sync.dma_start(
                agin2[:, :].rearrange("(a p) b -> p a b", p=128)[:, :, 0:32], HS2[:])
            nc.gpsimd.collective_compute(
                "AllGather", ALU.bypass, replica_groups=[list(range(NCORES))],
                ins=[agin2.opt()], outs=[agout2.opt()])

            # ---- layer 2 ----
            aggregate(agout2)
            nc.vector.tensor_add(T[:], T[:], HS2[:])
            nc.vector.tensor_mul(T[:], T[:], dinv_b)
            nc.vector.tensor_add(T[:], T[:], bias_b(B2))
            nc.scalar.activation(T[:], T[:], AF.Relu)

            # ---- head: global max pool + FC + log_softmax ----
            GMAX = per_p.tile([128, 32], dt.float32)
            nc.vector.tensor_copy(GMAX[:], T[:, 0, :])
            for t in range(1, NB):
                nc.vector.tensor_tensor(GMAX[:], GMAX[:], T[:, t, :], op=ALU.max)
            psg2 = psa_p.tile([32, 128], dt.float32, tag="ptr", bufs=1)
            nc.tensor.transpose(out=psg2[:], in_=GMAX[:], identity=IDN[:])
            GT = per_p.tile([32, 128], dt.float32)
            nc.scalar.copy(GT[:], psg2[:])
            GV = per_p.tile([32, 1], dt.float32)
            nc.vector.reduce_max(GV[:], GT[:], axis=AX.X)
            nc.sync.dma_start(arin[:, :], GV[:])
            nc.gpsimd.collective_compute(
                "AllReduce", ALU.max, replica_groups=[list(range(NCORES))],
                ins=[arin.opt()], outs=[arout.opt()])
            GAR = per_p.tile([32, 1], dt.float32)
            nc.sync.dma_start(GAR[:], arout[:, :])
            psl = psa_p.tile([1, 5], dt.float32, tag="plg", bufs=1)
            nc.tensor.matmul(psl[:], lhsT=GAR[:], rhs=FCW[:], start=True, stop=True)
            LG = per_p.tile([1, 5], dt.float32)
            nc.vector.tensor_add(LG[:], psl[:], FCB[:])
            MX = per_p.tile([1, 1], dt.float32)
            nc.vector.reduce_max(MX[:], LG[:], axis=AX.X)
            nc.vector.tensor_tensor(LG[:], LG[:], MX[:].to_broadcast([1, 5]),
                                    op=ALU.subtract)
            EX = per_p.tile([1, 5], dt.float32)
            nc.scalar.activation(EX[:], LG[:], AF.Exp)
            SM = per_p.tile([1, 1], dt.float32)
            nc.vector.reduce_sum(SM[:], EX[:], axis=AX.X)
            LS = per_p.tile([1, 1], dt.float32)
            nc.scalar.activation(LS[:], SM[:], AF.Ln)
            nc.vector.tensor_tensor(LG[:], LG[:], LS[:].to_broadcast([1, 5]),
                                    op=ALU.subtract)
            nc.sync.dma_start(out_t[:, :], LG[:])

    nc.compile()

    in_maps = []
    for k in range(NCORES):
        in_maps.append({
            "xsT": xsT, "xso": xsT_own[k], "g16": g16_all[k], "dlf": dlf_all[k],
            "dnv": dinv_sl[k], "iota": iota,
            "W1": np.asarray(W1, np.float32), "W2": np.asarray(W2, np.float32),
            "b1t": b1t, "b2t": b2t, "fcW": np.asarray(fcW, np.float32), "fcb": fcb2,
        })
    return nc, in_maps


def kernel(x, edge_index, W1, b1, W2, b2, fcW, fcb):
    global LAST_RESULTS, RUN_WALL_NS
    nc, in_maps = build(x, edge_index, W1, b1, W2, b2, fcW, fcb)
    import os, time as _time
    trace = os.environ.get("GNN_TRACE", "0") == "1"
    _t0 = _time.time()
    res = run_bass_kernel_spmd(nc, in_maps, core_ids=list(range(NCORES)), trace=trace)
    RUN_WALL_NS = int((_time.time() - _t0) * 1e9)
    LAST_RESULTS = res
    return res.results[0]["out"].astype(np.float32)
